# revision 23
# baseline (speedup 1.0000x reference)
"""
Trainium2 Bass kernel for nn_LinearLUT (residual-binarized LUT linear layer).

Math restructure (same algebra as the earlier baseline)
-------------------------------------------------------
reference(x) computes, per sample b and per table t (t = o*128 + j):

  table_out[b,t] = sum_l f_t(m_l * s_l[b, idx_1(t)], ..., m_l * s_l[b, idx_4(t)])

Since every argument is +-m_l, f_t only depends on the 4 sign bits =>
precompute (host, weight-static) a 16-entry lookup Q_l[t, v] indexed by
the 4-bit sign code  code_l[b,t] = (bit_l @ G)[b,t]  -- ONE matmul per level.

Step-basis LUT evaluation (Abel summation):
  Q_l[t, c] = Q_l[t, 0] + sum_{v=1..15} dQ_l[t,v] * [c >= v]
Each basis plane [c >= v] is ONE elementwise instruction on DVE (is_ge,
fp16 SBUF, 4x mode: 0.26 ns/col), ACT (Sign activation: 0.83 ns/col) or
POOL (is_ge: 0.83 ns/col).  v in {1,2,4,8} fold into the linear span
{1, b0..b3} and become one small matmul per level (W8).  The per-output
segment sum is fused into PSUM-accumulated N=1 matmuls (j-contraction).

Schedule (this version)
-----------------------
- Code matmuls run in 8 interleaved (q-chunk, level) groups so BOTH
  levels' codes reach SBUF fp16 early (the old baseline copied level 1
  as one late [128,2048] block, bunching ACT's planes after 7.7us).
- PSUM->SBUF fp16 copies are split between ACT and POOL per chunk.
- consts arrive via memset (no DMA); qcols/cvec DMAs issue from the
  POOL sequencer; one g chunk issues from the DVE sequencer -- the SP
  queue only carries xt + 3 g chunks (500ns serialized slices).
- A compile-time greedy planner orders each engine's plane ops by
  copy-availability (DVE ladders chunk-wise early, goes wide later) and
  orders the PE accumulation matmuls by estimated plane completion.
"""

import numpy as np

import concourse.bass as bass
import concourse.bacc as bacc
import concourse.mybir as mybir
import concourse.tile as tile
from concourse.bass_utils import run_bass_kernel_spmd

# Problem dims (hardcoded per contract)
LEVELS = 2
K = 4
KK = 16
IN = 128
OUT = 128
B = 128
T = IN * OUT  # 16384
NCORES = 8
T_C = T // NCORES     # 2048 tables per core
OL = OUT // NCORES    # 16 out features per core
NTILE = T_C // 128    # 16 t-tiles per core
NV = KK - 1           # 15 step thresholds v=1..15 (v=0 folded into cvec)

F16 = mybir.dt.float16
F32 = mybir.dt.float32

# ---------------- static schedule configuration (tunable) ----------------
# v in {1,2,4,8} are folded into the linear bit space (see _make_in_maps);
# the remaining 11 thresholds per level are computed as planes.
V_LIST = [3, 5, 6, 7, 9, 10, 11, 12, 13, 14, 15]
SKIP_PLANES = {(l, v) for l in range(2) for v in (1, 2, 4, 8)}

# plane -> engine ('D' = DVE is_ge 0/1, 'A' = ACT Sign +-1, 'P' = POOL is_ge)
# per-engine plane lists (priority order), entries (l, v, t_lo, t_hi).
# D is the fast engine (0.26 ns/col); A additionally carries all
# PSUM->SBUF copies (GPSIMD cannot touch PSUM on real HW, and DVE plane
# time is 3.2x more valuable than a copy), so A gets the fewest planes.
# Balance solve: T ~= 12.9us at a=2.75, p=4.85, d=14.4 plane-equivalents.
D_PLANES = ([(l, v, 0, 16) for v in (3, 5, 6, 7, 9, 10, 11)
             for l in (0, 1)]
            + [(0, 13, 12, 16), (1, 13, 14, 16)])
A_PLANES = [(1, 12, 0, 16), (0, 12, 0, 16), (0, 13, 0, 12)]
P_PLANES = [(1, 14, 0, 16), (0, 14, 0, 16), (1, 15, 0, 16),
            (0, 15, 0, 16), (1, 13, 0, 14)]

# code-matmul chunk order: (q, l); chunk q covers tiles [4q, 4q+4).
# All g chunks issue on SP after xt (500ns serialized slices).
CHUNK_ORDER = [(0, 0), (0, 1), (1, 0), (1, 1), (2, 0), (2, 1), (3, 0), (3, 1)]
# copies (all on ACT): (level, [q chunks]) -- contiguous q runs only
COPIES = [(0, [0]), (1, [0]), (0, [1, 2]), (1, [1, 2]), (0, [3]), (1, [3])]
# estimated g-chunk visibility (ns): g_q gates chunk (q, *) matmuls
_G_VIS = [2917.0, 3417.0, 3917.0, 4000.0]

# ---------------- planner cost model (ns, calibrated on CoreSim) ---------
_MM_START = 2917.0
_MM_NS = 53.0
_SEM = 50.0
_COPY_A = {512: 612.0, 1024: 1038.0, 2048: 1892.0}
_COPY_P = {512: 522.0, 1024: 948.0, 2048: 1802.0}


def _cost(eng, cols):
    if eng == "D":
        return cols * 0.2604 + 60.0
    if eng == "A":
        return cols * 0.8333 + 185.0
    return cols * 0.8333 + 95.0


def _plan():
    """Greedy compile-time event simulation.

    Returns (copy_sched, pieces):
      copy_sched: list of (l, [q..], est_done) in ACT emission order
      pieces: list of (l, v, eng, t_lo, t_hi, est_done); per-engine
        emission order is order of appearance; PE accum order is by
        est_done.  l == -1 denotes a both-level [128, 4096] op.
    """
    mm_done = {}
    t = _MM_START
    for (q, l) in CHUNK_ORDER:
        t = max(t, _G_VIS[q] + _SEM) + 4 * _MM_NS
        mm_done[(q, l)] = t

    # --- ACT: copies first, then its planes ---
    copy_sched = []
    avail = {}
    now_a = 3100.0
    for (l, qs) in COPIES:
        gate = max(mm_done[(q, l)] for q in qs) + _SEM
        now_a = max(now_a, gate) + _COPY_A.get(
            len(qs) * 512, len(qs) * 512 * 0.8333 + 185)
        for q in qs:
            avail[(l, q)] = now_a + _SEM
        copy_sched.append((l, qs, now_a))

    pieces = []

    def ladder(eng, plane_list, start_t):
        """Greedy ladder: repeatedly emit the widest available
        contiguous tile run over the engine's plane entries; both-level
        merge when two full entries share v and all chunks are ready."""
        rem = {i: set(range(lo, hi))
               for i, (l, v, lo, hi) in enumerate(plane_list)}
        touched = set()
        now = start_t
        guard = 0
        full_idx = {}
        for i, (l, v, lo, hi) in enumerate(plane_list):
            if lo == 0 and hi == NTILE:
                full_idx[(l, v)] = i

        def tile_ok(l, t):
            return avail.get((l, t // 4), 9e9) <= now

        while any(rem.values()) and guard < 3000:
            guard += 1
            best = None  # (i or ('B', i0, i1), v, t_lo, t_hi)
            best_w = 0
            for i, (l, v, lo, hi) in enumerate(plane_list):
                if not rem[i]:
                    continue
                if (l == 0 and (1, v) in full_idx and (0, v) in full_idx
                        and i == full_idx[(0, v)]
                        and i not in touched
                        and full_idx[(1, v)] not in touched
                        and rem[full_idx[(1, v)]]
                        and all(avail.get((ll, q), 9e9) <= now
                                for ll in (0, 1) for q in range(4))):
                    best = (("B", i, full_idx[(1, v)]), v, 0, NTILE)
                    best_w = 2 * NTILE
                    break
                run, brun = [], []
                for t in range(lo, hi):
                    if t in rem[i] and tile_ok(l, t):
                        run.append(t)
                        if len(run) > len(brun):
                            brun = list(run)
                    else:
                        run = []
                if len(brun) > best_w:
                    best = (i, v, brun[0], brun[-1] + 1)
                    best_w = len(brun)
            if best is None:
                pend = [a for a in avail.values() if a > now]
                if not pend:
                    break
                nxt = min(a for a in pend)
                if nxt <= now:
                    break
                now = nxt
                continue
            key, v, t_lo, t_hi = best
            now += _cost(eng, best_w * B)
            if isinstance(key, tuple):
                _, i0, i1 = key
                pieces.append((-1, v, eng, 0, NTILE, now))
                rem[i0] = set()
                rem[i1] = set()
                touched.update((i0, i1))
            else:
                plane_l = plane_list[key][0]
                pieces.append((plane_l, v, eng, t_lo, t_hi, now))
                rem[key] -= set(range(t_lo, t_hi))
                touched.add(key)
        return now

    ladder("D", D_PLANES, 2915.0)
    ladder("A", A_PLANES, now_a)
    ladder("P", P_PLANES, 3110.0)
    return copy_sched, pieces


def _engine_of(l, v, tile):
    """Engine computing plane (l, v) for tile `tile` (for host coeffs)."""
    _, pieces = _plan_cached()
    for (pl, pv, eng, t_lo, t_hi, _e) in pieces:
        if pv == v and (pl == -1 or (pl == l and t_lo <= tile < t_hi)):
            return eng
    return "D"


_PLAN_CACHE = None


def _plan_cached():
    global _PLAN_CACHE
    if _PLAN_CACHE is None:
        _PLAN_CACHE = _plan()
    return _PLAN_CACHE


_CACHED_NC = None
_CACHED_M0 = None
_DEBUG_PIECE_INSTS = []


def _build_nc(m0):
    """Build the per-core Bass program (identical on all 8 cores)."""
    nc = bacc.Bacc("TRN2", target_bir_lowering=False, debug=False,
                   num_devices=NCORES)

    copy_sched, pieces = _plan_cached()
    act_thrs = sorted({v for (_l, v, e, _lo, _hi, _t) in pieces
                       if e == "A"})
    xt = nc.dram_tensor("xt", [IN, B], F32, kind="ExternalInput")
    g = nc.dram_tensor("g", [IN, T_C], F16, kind="ExternalInput")
    qcols = nc.dram_tensor("qcols", [128, LEVELS * NTILE * NV + LEVELS * OL],
                           F16, kind="ExternalInput")
    cvec = nc.dram_tensor("cvec", [1, OL], F32, kind="ExternalInput")
    y = nc.dram_tensor("y", [B, OL], F32, kind="ExternalOutput")

    pe_order = sorted(range(len(pieces)), key=lambda i: pieces[i][5])
    # one eq buffer per piece (exact-size pools): no WAR hazards, so the
    # PE accum order can never deadlock against plane emission order.
    from collections import Counter
    piece_cols = []
    for (l, v, eng, t_lo, t_hi, _e) in pieces:
        piece_cols.append(LEVELS * NTILE * B if l < 0 else (t_hi - t_lo) * B)
    size_counts = Counter(piece_cols)

    import contextlib
    with tile.TileContext(nc) as tc:
        with contextlib.ExitStack() as stack:
            cpool = stack.enter_context(tc.tile_pool(name="const", bufs=1))
            bpool = stack.enter_context(tc.tile_pool(name="bits", bufs=1))
            csbpool = stack.enter_context(tc.tile_pool(name="codesb", bufs=1))
            eq_pools = {
                sz: stack.enter_context(
                    tc.tile_pool(name=f"eqp{sz}", bufs=cnt))
                for sz, cnt in sorted(size_counts.items())
            }
            opool = stack.enter_context(tc.tile_pool(name="out", bufs=1))
            pc = {
                512: stack.enter_context(tc.tile_pool(
                    name="psum_code512", bufs=3,
                    space=bass.MemorySpace.PSUM)),
                1024: stack.enter_context(tc.tile_pool(
                    name="psum_code1024", bufs=2,
                    space=bass.MemorySpace.PSUM)),
            }
            py = stack.enter_context(tc.tile_pool(
                name="psum_y", bufs=1, space=bass.MemorySpace.PSUM))
            xt_sb = cpool.tile([IN, B], F32, tag="xt")
            cv_sb = cpool.tile([1, OL], F32, tag="cvec")
            ones_sb = cpool.tile([1, B], F32, tag="ones")
            bias_sb = cpool.tile([128, max(1, len(act_thrs))], F32,
                                 tag="actbias")
            g_sb = [cpool.tile([IN, 512], F16, tag=f"g{q}", name=f"g_sb{q}")
                    for q in range(4)]
            q_sb = cpool.tile([128, LEVELS * NTILE * NV + LEVELS * OL], F16,
                              tag="qcols")
            # SP queue: xt first (gates bits), then g0..g3 (500ns slices)
            nc.sync.dma_start(xt_sb[:], xt[:])
            for q in range(4):
                nc.sync.dma_start(g_sb[q][:], g[:, q * 512:(q + 1) * 512])
            # qcols + cvec from the POOL sequencer (SWDGE; off the SP queue)
            nc.gpsimd.dma_start(cv_sb[:], cvec[:])
            nc.gpsimd.dma_start(q_sb[:], qcols[:])
            nc.gpsimd.memset(ones_sb[:], 1.0)
            # ACT Sign biases are compile-time: memset, no DMA
            for i, v in enumerate(act_thrs):
                nc.gpsimd.memset(bias_sb[:, i:i + 1], -(float(v) - 0.5))

            # ---- sign bits (as fp16 0/1, j on partitions) ----
            bit1 = bpool.tile([IN, B], F16, tag="bit1")
            nc.vector.tensor_scalar(bit1[:], xt_sb[:], 0.0, None,
                                    mybir.AluOpType.is_ge)
            # rc = x - 2*m0*bit1   (== resid - m0)
            rc = bpool.tile([IN, B], F32, tag="rc")
            nc.vector.scalar_tensor_tensor(rc[:], bit1[:], -2.0 * m0,
                                           xt_sb[:], mybir.AluOpType.mult,
                                           mybir.AluOpType.add)
            # bit2 = (rc >= -m0)
            bit2 = bpool.tile([IN, B], F16, tag="bit2")
            nc.vector.tensor_scalar(bit2[:], rc[:], -m0, None,
                                    mybir.AluOpType.is_ge)
            bits = [bit1, bit2]

            # ---- code matmuls (interleaved chunks) + fp16 copies (ACT) ----
            # codesb[t_p, l*2048 + tile*B + b]: one [128, 4096] fp16 tensor
            codesb = csbpool.tile([128, LEVELS * NTILE * B], F16, tag="code")
            # PSUM tiles are allocated per copy-group so multi-chunk copies
            # read one contiguous source.
            grp_of = {}
            for gi, (l, qs) in enumerate(COPIES):
                for q in qs:
                    grp_of[(l, q)] = gi
            grp_tiles = {}
            for (q, l) in CHUNK_ORDER:
                gi = grp_of[(l, q)]
                l_g, qs_g = COPIES[gi]
                if gi not in grp_tiles:
                    w = len(qs_g) * 512
                    grp_tiles[gi] = pc[w].tile(
                        [128, w], F32, tag=f"cps{w}", name=f"cps{l}_{gi}")
                cps = grp_tiles[gi]
                off = qs_g.index(q) * 512
                for k in range(4):
                    nc.tensor.matmul(
                        cps[:, off + k * B:off + (k + 1) * B],
                        g_sb[q][:, k * B:(k + 1) * B],
                        bits[l][:],
                        start=True, stop=True,
                    )
            for gi, (l, qs) in enumerate(COPIES):
                w = len(qs) * 512
                c0 = (l * NTILE + qs[0] * 4) * B
                nc.scalar.copy(codesb[:, c0:c0 + w], grp_tiles[gi][:])

            # ---- seed + linear (v in {1,2,4,8}) matmuls ----
            y_ps = py.tile([B, OL], F32, tag="ypsum")
            nc.tensor.matmul(y_ps[:], ones_sb[:], cv_sb[:],
                             start=True, stop=False)
            W8_BASE = LEVELS * NTILE * NV
            for l in range(LEVELS):
                nc.tensor.matmul(
                    y_ps[:],
                    bits[l][:],
                    q_sb[:, W8_BASE + l * OL:W8_BASE + (l + 1) * OL],
                    start=False, stop=False,
                )

            # ---- step planes, emitted per engine in planner order ----
            eq_tiles = [None] * len(pieces)
            for idx, (l, v, eng, t_lo, t_hi, _est) in enumerate(pieces):
                if l < 0:  # both-level piece: cols [0, 4096)
                    cols = LEVELS * NTILE * B
                    src = codesb[:, 0:cols]
                else:
                    cols = (t_hi - t_lo) * B
                    c0 = (l * NTILE + t_lo) * B
                    src = codesb[:, c0:c0 + cols]
                eq = eq_pools[cols].tile([128, cols], F16, tag=f"eq{cols}",
                                         name=f"eq_{idx}")
                eq_tiles[idx] = eq
                thr = float(v) - 0.5
                if eng == "D":
                    ins = nc.vector.tensor_scalar(eq[:, 0:cols], src, thr,
                                                  None, mybir.AluOpType.is_ge)
                elif eng == "P":
                    ins = nc.gpsimd.tensor_scalar(eq[:, 0:cols], src, thr,
                                                  None, mybir.AluOpType.is_ge)
                else:
                    bcol = act_thrs.index(v)
                    ins = nc.scalar.activation(
                        eq[:, 0:cols], src,
                        mybir.ActivationFunctionType.Sign,
                        bias=bias_sb[:, bcol:bcol + 1])
                _DEBUG_PIECE_INSTS.append((idx, pieces[idx],
                                           getattr(ins, "name", None)))

            # ---- fused LUT-eval + segment-sum: N=1 PSUM matmuls ----
            for rank, idx in enumerate(pe_order):
                l, v, eng, t_lo, t_hi, _est = pieces[idx]
                eq = eq_tiles[idx]
                last_piece = rank == len(pe_order) - 1
                if l < 0:
                    tiles = [(ll, t) for ll in range(LEVELS)
                             for t in range(NTILE)]
                else:
                    tiles = [(l, t) for t in range(t_lo, t_hi)]
                for j, (ll, t) in enumerate(tiles):
                    col = (ll * NTILE + t) * NV + (v - 1)
                    nc.tensor.matmul(
                        y_ps[:, t:t + 1],
                        eq[:, j * B:(j + 1) * B],
                        q_sb[:, col:col + 1],
                        start=False,
                        stop=(last_piece and j == len(tiles) - 1),
                    )

            y_sb = opool.tile([B, OL], F32, tag="ysb")
            nc.vector.tensor_copy(y_sb[:], y_ps[:])
            nc.sync.dma_start(y[:], y_sb[:])

    nc.compile()
    return nc


def _host_prep(x, weight, bias, means):
    """Weight-static preprocessing: Q LUTs per level (fp64)."""
    w = weight.astype(np.float64)
    m = np.abs(means.astype(np.float64))
    cc = np.arange(KK)
    tt = (2 * ((cc[:, None] >> np.arange(K)[None, :]) & 1) - 1).astype(
        np.float64)          # [c, i]
    sig = tt                  # same construction for sign patterns [v, i]

    qs = []
    for l in range(LEVELS):
        # M[v, c] = prod_i (1 + m_l * sig[v,i] * tt[c,i]) / 2
        M = np.prod((1.0 + m[l] * sig[:, None, :] * tt[None, :, :]) * 0.5,
                    axis=-1)  # [v, c]
        q = w @ M.T           # [T, KK]
        qs.append(q)
    return qs


def _build_g(input_mask):
    G = np.zeros((IN, T), np.float64)
    cols = np.repeat(np.arange(T), K)
    vals = np.tile(2.0 ** np.arange(K), T)
    np.add.at(G, (input_mask.astype(np.int64), cols), vals)
    return G


def _plane_tile_engine(l, v, tile):
    """Engine that computes basis plane (l, v) for tile `tile`."""
    return _engine_of(l, v, tile)


def _make_in_maps(x, weight, bias, means, input_mask):
    qs = _host_prep(x, weight, bias, means)
    G = _build_g(input_mask)

    xt = np.ascontiguousarray(x.astype(np.float32).T)

    # step-basis coefficients: dq[t, v] = Q[t, v] - Q[t, v-1], v=1..15.
    # DVE/POOL planes are 0/1 steps (coeff dq); ACT planes are +-1 signs
    # (coeff dq/2, plus dq/2 folded into the constant).
    tile_of = (np.arange(T) % T_C) // 128    # core-local tile index [T]
    dqs, c0s, lins = [], [], []
    for l in range(LEVELS):
        dq = np.diff(qs[l], axis=1)          # [T, 15]
        c0 = qs[l][:, 0].copy()              # [T]
        # fold v in {1,2,4,8} into the linear bit space (see SKIP_PLANES):
        d1, d2 = dq[:, 0].copy(), dq[:, 1].copy()
        d4, d8 = dq[:, 3].copy(), dq[:, 7].copy()
        for u in (3, 5, 7, 9, 11, 13, 15):   # T1 correction
            dq[:, u - 1] -= d1
        for u in (6, 10, 14):                # T2 correction
            dq[:, u - 1] -= d2
        dq[:, 12 - 1] -= d4                  # T4 correction
        dq[:, [0, 1, 3, 7]] = 0.0
        # per-table gather coefficients for (idx_0..idx_3)
        lins.append((d1, d1 + d2, 2 * d1 + d2 + d4,
                     4 * d1 + 2 * d2 + d4 + d8))
        coeff = dq.copy()
        for v in range(1, KK):
            is_a = np.array([_plane_tile_engine(l, v, ti) == "A"
                             for ti in range(NTILE)])[tile_of]
            coeff[:, v - 1] = np.where(is_a, dq[:, v - 1] * 0.5,
                                       dq[:, v - 1])
            c0 += np.where(is_a, dq[:, v - 1] * 0.5, 0.0)
        dqs.append(coeff)
        c0s.append(c0)

    # const[o] = bias[o] + sum_l sum_j c0_l[o*IN+j]
    cvec_full = bias.astype(np.float64).copy()
    for l in range(LEVELS):
        cvec_full += c0s[l].reshape(OUT, IN).sum(-1)

    in_maps = []
    idxs = input_mask.astype(np.int64).reshape(T, K)
    for c in range(NCORES):
        t0 = c * T_C
        sl = slice(t0, t0 + T_C)
        gc = G[:, sl].astype(np.float16)
        # qcols[j, (l, tile, v-1)] = coeff_l[t0 + tile*128 + j, v]
        qc = np.empty((128, LEVELS, NTILE, NV), np.float16)
        # Wlin_l[jin, o]: linear bit-gather coefficients per idx position
        wlin = np.zeros((128, LEVELS, OL), np.float64)
        tcol = np.arange(T_C) // 128
        for l in range(LEVELS):
            qc[:, l] = dqs[l][sl].reshape(
                NTILE, 128, NV).transpose(1, 0, 2)
            for i in range(K):
                np.add.at(wlin[:, l], (idxs[sl, i], tcol), lins[l][i][sl])
        qcols_full = np.concatenate(
            [qc.reshape(128, -1),
             wlin.reshape(128, LEVELS * OL).astype(np.float16)], axis=1)
        in_maps.append({
            "xt": xt,
            "g": np.ascontiguousarray(gc),
            "qcols": np.ascontiguousarray(qcols_full),
            "cvec": np.ascontiguousarray(
                cvec_full[c * OL:(c + 1) * OL].astype(np.float32)[None, :]),
        })
    return in_maps


_LAST_RESULTS = None


def kernel(x, weight, bias, means, input_mask):
    global _CACHED_NC, _CACHED_M0, _LAST_RESULTS
    m0 = float(np.abs(np.asarray(means, dtype=np.float64))[0])
    if _CACHED_NC is None or _CACHED_M0 != m0:
        _CACHED_NC = _build_nc(m0)
        _CACHED_M0 = m0
    nc = _CACHED_NC

    in_maps = _make_in_maps(x, weight, bias, means, input_mask)
    res = run_bass_kernel_spmd(nc, in_maps, list(range(NCORES)))
    _LAST_RESULTS = res
    out = np.concatenate([res.results[c]["y"] for c in range(NCORES)], axis=1)
    return out.astype(np.float32)
